# revision 40
# baseline (speedup 1.0000x reference)
"""Trainium2 Bass kernel for a dense transformer block.

Block: y = x + proj(MHA(LN1(x), rel-pos-bias)) ; out = y + fc2(gelu(fc1(LN2(y))))
Shapes (hardcoded): B=4, N=2048, C=512, H=8, DH=64, HID=2048, fp32 I/O.

Sharding over 8 cores: core c -> (batch b = c//2, query-half par = c%2).
Each core receives its batch's rows rolled so its own 1024 query tokens come
first, computes K/V over all 2048 tokens (duplicated across the pair of cores
sharing a batch -- cheaper than a collective), and runs attention + MLP for its
own 1024 tokens. Weights are replicated; LayerNorm affine params are folded
into the matmul weights on the host.

On-chip layout: activations are kept transposed ([C, tok]) so that
  - scores    S^T[k,q]  = matmul(lhsT=K^T slice, rhs=Q^T slice)   (per head)
  - attnV     O^T[dh,q] = matmul(lhsT=V_aug[k,65], rhs=P^T[k,q])  (accum over k)
both run the PE at full rate. All matmul operands are bf16 (PE accumulates in
fp32); the residual stream, LN stats, scores (pre-exp) and softmax denominator
stay fp32. Bias rows and the softmax reciprocal enter matmuls as hi+lo bf16
pairs so their contribution is fp32-accurate. V is augmented with a ones
column so the attnV accumulation also produces the softmax denominator
(row 64). The relative position bias is served from host-precomputed shifted
tables so the bias tile for any (head, k-tile) is a plain free-dim slice of a
[128, 1920] SBUF block.

Engine balance (this revision): the softmax PSUM-evacuate+bias adds are the
dominant elementwise cost; they are split between the Vector (DVE) and Pool
(GpSimd) engines per-chunk, the exp runs on ACT over 4-k-tile batches
([128, 4, 1024] per op) to amortize per-op overhead, and the K/Q/fc1 bias
adds ride the ACT activation op's per-partition bias operand straight out of
PSUM. Attention output stays in SBUF across phases (no DRAM roundtrip).
"""

import threading
from contextlib import ExitStack

import numpy as np

import concourse.bass as bass
import concourse.tile as tile
from concourse import bacc, mybir
from concourse.bass_utils import run_bass_kernel_spmd
from concourse.masks import make_identity

F32 = mybir.dt.float32
BF16 = mybir.dt.bfloat16
FP8 = mybir.dt.float8e4
DR = mybir.MatmulPerfMode.DoubleRow
WS = 64.0            # fp8 weight pre-scale (undone in the PSUM evacuations)

B, N, C, H = 4, 2048, 512, 8
DH = C // H          # 64
HID = 4 * C          # 2048
NQ = N // 2          # own query tokens per core (1024)
EPS = 1e-5
P = 128              # partitions
TT = N // P          # 16 token tiles (full batch)
TQ = NQ // P         # 8 token tiles (own)
CT = C // P          # 4 channel tiles
OT = HID // P        # 16 hidden tiles
BLKW = NQ + 7 * P    # 1920, bias block width
KB = 2               # k-tiles per softmax batch (exp granularity)
# heads whose softmax runs multiplicatively (ACT exp from PSUM + Pool mult by
# host-precomputed exp(bias)); the rest use additive bias blocks
M_HEADS = frozenset({5, 6, 7})

def build_program(reps: int = 1, phases: str = "abcde"):
    """Build the per-core Bass program (SPMD; all per-core differences are
    carried by input data)."""
    nc = bacc.Bacc("TRN2", target_bir_lowering=False, debug=False, num_devices=8)

    t = {}
    t["xb"] = nc.dram_tensor("xb", [N, C], F32, kind="ExternalInput").ap()
    # fp8 DoubleRow-packed weights: [p, pair, row-in-pair, out] with
    # contraction row  c = (pair*2 + jj)*128 + p,  values pre-scaled by WS
    t["wqkvT"] = nc.dram_tensor("wqkvT", [P, 2, 2, 3 * C], FP8,
                                kind="ExternalInput").ap()
    t["bqk"] = nc.dram_tensor("bqk", [2 * C], F32, kind="ExternalInput").ap()
    t["bv2"] = nc.dram_tensor("bv2", [2, C], BF16, kind="ExternalInput").ap()
    t["wprojT"] = nc.dram_tensor("wprojT", [P, 2, 2, C], FP8,
                                 kind="ExternalInput").ap()
    t["bproj2"] = nc.dram_tensor("bproj2", [2, C], BF16,
                                 kind="ExternalInput").ap()
    t["wfc1T"] = nc.dram_tensor("wfc1T", [P, 2, 2, HID], FP8,
                                kind="ExternalInput").ap()
    t["bfc1"] = nc.dram_tensor("bfc1", [HID], F32, kind="ExternalInput").ap()
    t["wfc2T"] = nc.dram_tensor("wfc2T", [P, 8, 2, C], FP8,
                                kind="ExternalInput").ap()
    t["bfc22"] = nc.dram_tensor("bfc22", [2, C], BF16,
                                kind="ExternalInput").ap()
    t["blka"] = nc.dram_tensor("blka", [H, P, BLKW], BF16,
                               kind="ExternalInput").ap()
    t["blkb"] = nc.dram_tensor("blkb", [H, P, BLKW], BF16,
                               kind="ExternalInput").ap()
    t["dbg"] = nc.dram_tensor("dbg", [C, N], BF16).ap()
    t["phases"] = phases
    t["out"] = nc.dram_tensor("out", [NQ, C], F32, kind="ExternalOutput").ap()

    with tile.TileContext(nc) as tc:
        if reps == 1:
            _build_body(nc, tc, t)
        else:
            with tc.For_i(0, reps, 1):
                _build_body(nc, tc, t)
    nc.compile()
    return nc


def _kq_evac(nc, Alu, Act, out, ps, bcol, ctr):
    """PSUM evac with 1/WS rescale + per-partition bias, engine rotated."""
    n = ctr[1]
    ctr[1] += 1
    if n % 2 == 0:
        nc.scalar.activation(out=out, in_=ps, func=Act.Identity,
                             bias=bcol, scale=1.0 / WS)
    else:
        nc.vector.tensor_scalar(out=out, in0=ps, scalar1=1.0 / WS,
                                scalar2=bcol, op0=Alu.mult, op1=Alu.add)


def _build_body(nc, tc, t):
    Act = mybir.ActivationFunctionType
    Alu = mybir.AluOpType

    xb, out = t["xb"], t["out"]
    evac_n = [0, 0]

    with ExitStack() as ctx:
        singles = ctx.enter_context(tc.tile_pool(name="singles", bufs=1))
        ident = singles.tile([P, P], F32)
        make_identity(nc, ident)
        identb = singles.tile([P, P], BF16)
        nc.vector.tensor_copy(out=identb, in_=ident)
        eps_t = singles.tile([P, 1], F32)
        nc.gpsimd.memset(eps_t, EPS)
        ones2 = singles.tile([2, P], BF16)
        nc.gpsimd.memset(ones2, 1.0)

        x_own = [None] * TQ
        kT = [None] * CT
        qT = [None] * CT
        va = [None] * TT

        ad = ctx.enter_context(ExitStack())   # spans phases A..D
        xq_pool = ad.enter_context(tc.tile_pool(name="xq", bufs=1))
        oT_pool = ad.enter_context(tc.tile_pool(name="oT", bufs=2))
        # DoubleRow pair layout: oTp[pr][p, jj, tok] = O^T[(pr*2+jj)*128+p, :]
        oTp = [oT_pool.tile([P, 2, NQ], FP8, tag="oT", name=f"oTp{i}")
               for i in range(2)]
        ac_scope = ad.enter_context(ExitStack())  # spans phases A..C
        kT_pool = ac_scope.enter_context(tc.tile_pool(name="kT", bufs=CT))
        qT_pool = ac_scope.enter_context(tc.tile_pool(name="qT", bufs=CT))
        va_pool = ac_scope.enter_context(tc.tile_pool(name="va", bufs=TT))

        # ------------------------------------------------------------------
        # Phases A+B: LN1 (transposed) and QKV projections
        # ------------------------------------------------------------------
        with ExitStack() as ab:
            z1t_pool = ab.enter_context(tc.tile_pool(name="z1t", bufs=1))
            xload_pool = ab.enter_context(tc.tile_pool(name="xload", bufs=3))
            zt_pool = ab.enter_context(tc.tile_pool(name="zt", bufs=4))
            stat_pool = ab.enter_context(tc.tile_pool(name="stat", bufs=4))
            wq_pool = ab.enter_context(tc.tile_pool(name="wq", bufs=1))
            bias_pool = ab.enter_context(tc.tile_pool(name="qkvb", bufs=1))
            bpsum = ab.enter_context(
                tc.tile_pool(name="bpsum", bufs=4, space="PSUM"))

            # z1t[p, ct, tok] = z1[ct*128 + p, tok] (transposed, ct-tiled)
            z1t = z1t_pool.tile([P, CT, N], BF16, tag="z1t", name="z1t")

            # --- prefetch: x tiles (first two alone for fast LN start, then
            # grouped), QKV weights as one strided DMA, then biases — all
            # back-to-back on the SP queue with no data-dependent DMAs in
            # between, so loads stream at full bandwidth ---
            xq_big = xq_pool.tile([P, TQ, C], F32, tag="xqb", name="xqb")
            xl_big = xq_pool.tile([P, TT - TQ, C], F32, tag="xlb", name="xlb")
            x_all = [xq_big[:, i, :] for i in range(TQ)] + \
                    [xl_big[:, i, :] for i in range(TT - TQ)]
            for tt in range(TQ):
                x_own[tt] = x_all[tt]
            xv = xb.rearrange("(t p) c -> p t c", p=P)
            nc.sync.dma_start(out=xq_big[:, 0, :], in_=xv[:, 0, :])
            nc.sync.dma_start(out=xq_big[:, 1, :], in_=xv[:, 1, :])
            wqall = wq_pool.tile([P, 2, 2, 3 * C], FP8, tag="wq",
                                 name="wqall")
            nc.sync.dma_start(out=wqall, in_=t["wqkvT"])
            nc.sync.dma_start(out=xq_big[:, 2:4, :], in_=xv[:, 2:4, :])
            nc.sync.dma_start(out=xq_big[:, 4:TQ, :], in_=xv[:, 4:TQ, :])
            nc.sync.dma_start(out=xl_big[:, 0:4, :], in_=xv[:, TQ:TQ + 4, :])
            nc.sync.dma_start(out=xl_big[:, 4:8, :],
                              in_=xv[:, TQ + 4:TT, :])
            # fp32 bias columns for Q (o 0..3) and K (o 4..7), one DMA
            bqk8 = bias_pool.tile([P, 8], F32, tag="bcol")
            nc.sync.dma_start(
                out=bqk8, in_=t["bqk"].rearrange("(o p) -> p o", p=P))
            bcols = [bqk8[:, ot:ot + 1] for ot in range(8)]
            bvrow = bias_pool.tile([2, C], BF16, tag="bvrow")
            nc.sync.dma_start(out=bvrow, in_=t["bv2"])

            # --- A: LayerNorm1 (stats in natural layout; normalize on the
            # idle Pool; xbar-transpose DMAs staggered on the SP queue so a
            # pending transpose never delays independent loads) ---
            zs = [None] * TT

            def emit_transpose(tt):
                nc.sync.dma_start_transpose(
                    out=z1t[:, :, tt * P:(tt + 1) * P], in_=zs[tt])

            for tt in range(TT):
                x_t = x_all[tt]
                st = stat_pool.tile([P, 6], F32, tag="st")
                mv = stat_pool.tile([P, 2], F32, tag="mv")
                nc.vector.bn_stats(out=st, in_=x_t)
                nc.vector.bn_aggr(out=mv, in_=st)
                rs = stat_pool.tile([P, 1], F32, tag="rs")
                nc.scalar.activation(out=rs, in_=mv[:, 1:2], func=Act.Sqrt,
                                     bias=eps_t, scale=1.0)
                nc.vector.reciprocal(out=rs, in_=rs)
                z_t = zt_pool.tile([P, C], BF16, tag="zt", bufs=6,
                                   name=f"z{tt}")
                nc.gpsimd.tensor_scalar(out=z_t, in0=x_t, scalar1=mv[:, 0:1],
                                        scalar2=rs, op0=Alu.subtract,
                                        op1=Alu.mult)
                zs[tt] = z_t
                if tt >= 2:
                    emit_transpose(tt - 2)
            for tt in range(TT - 2, TT):
                emit_transpose(tt)

            # fp8 copy of z1t in DoubleRow pair layout, chunked by the token
            # groups the transposes produce (engines alternated)
            z1f8 = z1t_pool.tile([P, 2, 2, N], FP8, tag="z1f8", name="z1f8")
            for r in range(4):
                sl = slice(r * 512, (r + 1) * 512)
                eng = (nc.vector, nc.scalar, nc.vector, nc.scalar)[r]
                if eng is nc.scalar:
                    nc.scalar.activation(out=z1f8[:, :, :, sl],
                                         in_=z1t[:, :, sl], func=Act.Identity)
                else:
                    eng.tensor_copy(out=z1f8[:, :, :, sl], in_=z1t[:, :, sl])

            # --- B: QKV (fp8 DoubleRow matmuls; 1/WS rescale fused into
            # every PSUM evacuation) ---

            # V natural [tok, 512] + ones column per head -> [P, H, 65]
            for tt in range(TT):
                v_t = va_pool.tile([P, H * (DH + 1)], BF16, tag="va")
                va[tt] = v_t
                nc.gpsimd.memset(v_t, 1.0)
                ps = bpsum.tile([P, 512], F32, tag="mm")
                for pr in range(2):
                    nc.tensor.matmul(
                        ps,
                        lhsT=z1f8[:, pr, :, tt * P:(tt + 1) * P],
                        rhs=wqall[:, pr, :, 2 * C:3 * C],
                        start=(pr == 0), stop=False, perf_mode=DR)
                nc.tensor.matmul(ps, lhsT=ones2, rhs=bvrow,
                                 start=False, stop=True)
                nc.vector.tensor_scalar(
                    out=v_t.rearrange("p (h w) -> p h w", w=DH + 1)[:, :, 0:DH],
                    in0=ps.rearrange("p (h w) -> p h w", w=DH),
                    scalar1=1.0 / WS, scalar2=None, op0=Alu.mult)

            # K^T: heads along partitions (o-tiles 4..7 of qkv), all N tokens
            for ot in range(CT):
                k_t = kT_pool.tile([P, N], BF16, tag="kT")
                kT[ot] = k_t
                for tch in range(N // 512):
                    ps = bpsum.tile([P, 512], F32, tag="mm")
                    for pr in range(2):
                        nc.tensor.matmul(
                            ps,
                            lhsT=wqall[:, pr, :,
                                       C + ot * P:C + (ot + 1) * P],
                            rhs=z1f8[:, pr, :, tch * 512:(tch + 1) * 512],
                            start=(pr == 0), stop=(pr == 1), perf_mode=DR)
                    _kq_evac(nc, Alu, Act, k_t[:, tch * 512:(tch + 1) * 512],
                             ps, bcols[4 + ot], evac_n)
            # Q^T: o-tiles 0..3, own tokens only (first NQ columns of z1t)
            for ot in range(CT):
                q_t = qT_pool.tile([P, NQ], BF16, tag="qT")
                qT[ot] = q_t
                for tch in range(NQ // 512):
                    ps = bpsum.tile([P, 512], F32, tag="mm")
                    for pr in range(2):
                        nc.tensor.matmul(
                            ps,
                            lhsT=wqall[:, pr, :, ot * P:(ot + 1) * P],
                            rhs=z1f8[:, pr, :, tch * 512:(tch + 1) * 512],
                            start=(pr == 0), stop=(pr == 1), perf_mode=DR)
                    _kq_evac(nc, Alu, Act, q_t[:, tch * 512:(tch + 1) * 512],
                             ps, bcols[ot], evac_n)
        if "c" not in t["phases"]:
            for ot in range(CT):
                nc.sync.dma_start(out=t["dbg"][ot * P:(ot + 1) * P, :],
                                  in_=kT[ot])
                nc.sync.dma_start(out=t["dbg"][ot * P:(ot + 1) * P, 0:NQ],
                                  in_=qT[ot])
            for tt in range(TT):
                nc.sync.dma_start(
                    out=t["dbg"][0:P, tt * P:(tt + 1) * P],
                    in_=va[tt][:, 0:P])
            for tq in range(TQ):
                o_t = xq_pool.tile([P, C], F32, tag="dumout", bufs=2)
                nc.vector.tensor_copy(out=o_t, in_=x_own[tq])
                nc.sync.dma_start(out=out[tq * P:(tq + 1) * P, :], in_=o_t)
            return

        # ------------------------------------------------------------------
        # Phase C: attention, head-pairs, flash-style over batches of KB
        # k-tiles. Per super-batch: scores -> mixed DVE/Pool evac+bias ->
        # one big ACT exp -> attnV (pipelined one batch behind).
        # ------------------------------------------------------------------
        NSUP = TT // KB
        with ExitStack() as cx:
            blk_pool = cx.enter_context(tc.tile_pool(name="blk", bufs=4))
            sf_pool = cx.enter_context(tc.tile_pool(name="sf", bufs=3))
            t_pool = cx.enter_context(tc.tile_pool(name="texp", bufs=4))
            d_pool = cx.enter_context(tc.tile_pool(name="den", bufs=2))
            spsum = cx.enter_context(
                tc.tile_pool(name="spsum", bufs=2, space="PSUM"))
            acpsum = cx.enter_context(
                tc.tile_pool(name="acpsum", bufs=2, space="PSUM"))

            for hp in range(H // 2):
                h0, h1 = 2 * hp, 2 * hp + 1
                blks = {}
                for half, src in ((0, t["blka"]), (1, t["blkb"])):
                    bb = blk_pool.tile([P, 2, BLKW], BF16, tag="blk",
                                       name=f"blk{hp}_{half}")
                    nc.sync.dma_start(
                        out=bb,
                        in_=src[h0:h0 + 2].rearrange("h p w -> p h w"))
                    blks[(0, half)] = bb[:, 0, :]
                    blks[(1, half)] = bb[:, 1, :]
                ac = [acpsum.tile([DH + 1, NQ], F32, tag="ac",
                                  name=f"ac{hp}_{i}")
                      for i in range(2)]
                # software-pipelined by one super-batch: attnV matmuls for
                # batch s are emitted after batch s+1's scores+exp so the
                # in-order PE stream never blocks on the softmax chain.
                pend = []
                for sup in range(NSUP):
                    cur = []
                    for loc, hh in ((0, h0), (1, h1)):
                        mhead = hh in M_HEADS
                        sf = sf_pool.tile([P, KB, NQ], BF16, tag="sf",
                                          name=f"sf{hp}_{sup}_{loc}")
                        texp = t_pool.tile([P, KB, NQ], BF16, tag="texp",
                                           name=f"texp{hp}_{sup}_{loc}")
                        arows = []
                        for i in range(KB):
                            kt = sup * KB + i
                            half = 0 if kt < 8 else 1
                            off = (7 - kt % 8) * P
                            blk = blks[(loc, half)]
                            # p-variant: fold the bias into the PSUM via an
                            # extra identity matmul (PE has slack), then exp
                            # straight from PSUM; Pool can't touch PSUM.
                            pvar = (not mhead) and evac_n[0] % 7 == 6
                            evac_n[0] += 1
                            # [128,1024] fp32 psum (2 banks), 2 matmuls
                            sp = spsum.tile([P, NQ], F32, tag="sc",
                                            name=f"sc{hp}_{kt}_{loc}")
                            for qc in range(NQ // 512):
                                sl = slice(qc * 512, (qc + 1) * 512)
                                nc.tensor.matmul(
                                    sp[:, sl],
                                    lhsT=kT[hp][loc * DH:(loc + 1) * DH,
                                                kt * P:(kt + 1) * P],
                                    rhs=qT[hp][loc * DH:(loc + 1) * DH, sl],
                                    start=True, stop=(not pvar))
                                if pvar:
                                    nc.tensor.matmul(
                                        sp[:, sl], lhsT=identb,
                                        rhs=blk[:, off + qc * 512:
                                                off + (qc + 1) * 512],
                                        start=False, stop=True)
                            if mhead:
                                # exp from PSUM; this head's blocks hold
                                # exp(bias), multiplied in on the Pool engine
                                nc.scalar.activation(
                                    out=sf[:, i, :], in_=sp, func=Act.Exp,
                                    scale=float(DH) ** -0.5)
                                nc.gpsimd.tensor_tensor(
                                    out=texp[:, i, :], in0=sf[:, i, :],
                                    in1=blk[:, off:off + NQ], op=Alu.mult)
                            elif pvar:
                                nc.scalar.activation(
                                    out=texp[:, i, :], in_=sp, func=Act.Exp,
                                    scale=float(DH) ** -0.5)
                            else:
                                # a-variant: DVE evacuate + bias add; exp'd
                                # in contiguous batches below
                                nc.vector.tensor_tensor(
                                    out=sf[:, i, :], in0=sp,
                                    in1=blk[:, off:off + NQ], op=Alu.add)
                                arows.append(i)
                        # batch-exp the contiguous a-variant rows
                        r = 0
                        while r < len(arows):
                            r2 = r
                            while (r2 + 1 < len(arows)
                                   and arows[r2 + 1] == arows[r2] + 1):
                                r2 += 1
                            i0, i1 = arows[r], arows[r2] + 1
                            nc.scalar.activation(
                                out=texp[:, i0:i1, :], in_=sf[:, i0:i1, :],
                                func=Act.Exp, scale=float(DH) ** -0.5)
                            r = r2 + 1
                        cur.append((loc, hh, texp))
                    for loc, hh, texp in pend:
                        for i in range(KB):
                            kt = (sup - 1) * KB + i
                            for qc in range(NQ // 512):
                                sl = slice(qc * 512, (qc + 1) * 512)
                                nc.tensor.matmul(
                                    ac[loc][:, sl],
                                    lhsT=va[kt][:, hh * (DH + 1):
                                                (hh + 1) * (DH + 1)],
                                    rhs=texp[:, i, sl],
                                    start=(kt == 0), stop=False)
                    pend = cur
                for loc, hh, texp in pend:
                    for i in range(KB):
                        kt = (NSUP - 1) * KB + i
                        for qc in range(NQ // 512):
                            sl = slice(qc * 512, (qc + 1) * 512)
                            nc.tensor.matmul(
                                ac[loc][:, sl],
                                lhsT=va[kt][:, hh * (DH + 1):
                                            (hh + 1) * (DH + 1)],
                                rhs=texp[:, i, sl],
                                start=False, stop=(i == KB - 1))
                # normalize: O^T = num * (1/den); 1/den broadcast fp32 across
                # the DH partitions on Pool, multiplied on DVE straight into
                # the SBUF-resident fp8 pair-layout oT tiles.
                for loc in range(2):
                    hh = 2 * hp + loc
                    octile = oTp[hh // 4]
                    orow = (hh % 2) * DH
                    oj = (hh // 2) % 2
                    rden = d_pool.tile([1, NQ], F32, tag="rden")
                    nc.vector.reciprocal(out=rden, in_=ac[loc][DH:DH + 1, :])
                    rdenb = d_pool.tile([DH, NQ], F32, tag="rdenb")
                    nc.gpsimd.partition_broadcast(rdenb, rden)
                    nc.vector.tensor_tensor(
                        out=octile[orow:orow + DH, oj, :],
                        in0=ac[loc][0:DH, :], in1=rdenb,
                        op=Alu.mult)
        ac_scope.close()  # free kT/qT/va before phases D/E need SBUF

        if "d" not in t["phases"]:
            for tq in range(TQ):
                o_t = xq_pool.tile([P, C], F32, tag="dumout", bufs=2)
                nc.vector.tensor_copy(out=o_t, in_=x_own[tq])
                nc.sync.dma_start(out=out[tq * P:(tq + 1) * P, :], in_=o_t)
            return

        # ------------------------------------------------------------------
        # Phase D: proj + residual + LN2 (transposed)
        # ------------------------------------------------------------------
        x2_pool = ctx.enter_context(tc.tile_pool(name="x2", bufs=TQ))
        z2t_pool = ctx.enter_context(tc.tile_pool(name="z2t", bufs=1))
        # z2t[p, ct, tok] = z2[ct*128 + p, tok] (transposed, ct-tiled)
        z2t = z2t_pool.tile([P, CT, NQ], BF16, tag="z2t", name="z2t")
        x2 = [None] * TQ
        with ExitStack() as dx:
            wp_pool = dx.enter_context(tc.tile_pool(name="wp", bufs=1))
            brow_pool = dx.enter_context(tc.tile_pool(name="brow", bufs=1))
            stat2_pool = dx.enter_context(tc.tile_pool(name="stat2", bufs=4))
            ztmp_pool = dx.enter_context(tc.tile_pool(name="ztmp", bufs=3))
            dpsum = dx.enter_context(
                tc.tile_pool(name="dpsum", bufs=2, space="PSUM"))

            wpall = wp_pool.tile([P, 2, 2, C], FP8, tag="wp", name="wpall")
            nc.sync.dma_start(out=wpall, in_=t["wprojT"])
            bprow = brow_pool.tile([2, C], BF16, tag="bprow")
            nc.sync.dma_start(out=bprow, in_=t["bproj2"])

            z2s = [None] * TQ

            def emit_transpose2(tq):
                nc.sync.dma_start_transpose(
                    out=z2t[:, :, tq * P:(tq + 1) * P], in_=z2s[tq])

            for tq in range(TQ):
                ps = dpsum.tile([P, C], F32, tag="mm")
                for pr in range(2):
                    nc.tensor.matmul(
                        ps, lhsT=oTp[pr][:, :, tq * P:(tq + 1) * P],
                        rhs=wpall[:, pr, :, :], start=(pr == 0), stop=False,
                        perf_mode=DR)
                nc.tensor.matmul(ps, lhsT=ones2, rhs=bprow,
                                 start=False, stop=True)
                x2_t = x2_pool.tile([P, C], F32, tag="x2")
                x2[tq] = x2_t
                nc.vector.scalar_tensor_tensor(
                    out=x2_t, in0=ps, scalar=1.0 / WS, in1=x_own[tq],
                    op0=Alu.mult, op1=Alu.add)
                # LN2
                st = stat2_pool.tile([P, 6], F32, tag="st2")
                mv = stat2_pool.tile([P, 2], F32, tag="mv2")
                nc.vector.bn_stats(out=st, in_=x2_t)
                nc.vector.bn_aggr(out=mv, in_=st)
                rs = stat2_pool.tile([P, 1], F32, tag="rs2")
                nc.scalar.activation(out=rs, in_=mv[:, 1:2], func=Act.Sqrt,
                                     bias=eps_t, scale=1.0)
                nc.vector.reciprocal(out=rs, in_=rs)
                z_t = ztmp_pool.tile([P, C], BF16, tag="z2tmp", bufs=4,
                                     name=f"z2s{tq}")
                nc.gpsimd.tensor_scalar(out=z_t, in0=x2_t, scalar1=mv[:, 0:1],
                                        scalar2=rs, op0=Alu.subtract,
                                        op1=Alu.mult)
                z2s[tq] = z_t
                if tq >= 2:
                    emit_transpose2(tq - 2)
            for tq in range(TQ - 2, TQ):
                emit_transpose2(tq)

        # ------------------------------------------------------------------
        # Phase E: MLP
        # ------------------------------------------------------------------
        with ExitStack() as ex:
            w1_pool = ex.enter_context(tc.tile_pool(name="w1", bufs=1))
            g_pool = ex.enter_context(tc.tile_pool(name="g", bufs=OT))
            w2_pool = ex.enter_context(tc.tile_pool(name="w2", bufs=1))
            b1_pool = ex.enter_context(tc.tile_pool(name="b1", bufs=1))
            out_pool = ex.enter_context(tc.tile_pool(name="outp", bufs=2))
            epsum = ex.enter_context(
                tc.tile_pool(name="epsum", bufs=2, space="PSUM"))
            f2psum = ex.enter_context(
                tc.tile_pool(name="f2psum", bufs=4, space="PSUM"))

            w1all = w1_pool.tile([P, 2, 2, HID], FP8, tag="w1", name="w1all")
            nc.sync.dma_start(out=w1all, in_=t["wfc1T"])
            b1all = b1_pool.tile([P, OT], F32, tag="b1c")
            nc.sync.dma_start(
                out=b1all, in_=t["bfc1"].rearrange("(o p) -> p o", p=P))
            b1cols = [b1all[:, ot:ot + 1] for ot in range(OT)]
            b2row = b1_pool.tile([2, C], BF16, tag="b2row")
            nc.sync.dma_start(out=b2row, in_=t["bfc22"])
            w2all = w2_pool.tile([P, 8, 2, C], FP8, tag="w2", name="w2all")
            nc.sync.dma_start(out=w2all, in_=t["wfc2T"])

            # fp8 copy of z2t in DoubleRow pair layout
            z2f8 = z2t_pool.tile([P, 2, 2, NQ], FP8, tag="z2f8", name="z2f8")
            for r in range(2):
                sl = slice(r * 512, (r + 1) * 512)
                eng = (nc.vector, nc.scalar)[r]
                if eng is nc.scalar:
                    nc.scalar.activation(out=z2f8[:, :, :, sl],
                                         in_=z2t[:, :, sl], func=Act.Identity)
                else:
                    eng.tensor_copy(out=z2f8[:, :, :, sl], in_=z2t[:, :, sl])

            # fc1 + gelu produce gT in fp8 DoubleRow pair layout; fc2
            # accumulates incrementally per wave of 4 tq so it doesn't
            # serialize behind the whole fc1 loop.
            gTp = [None] * (OT // 2)
            WAVE = 4
            for half in range(TQ // WAVE):
                f2ps = [f2psum.tile([P, C], F32, tag="mm2",
                                    name=f"f2ps{half}_{w}")
                        for w in range(WAVE)]
                for ot in range(OT):
                    gpr, gj = ot // 2, ot % 2
                    if half == 0:
                        if gj == 0:
                            gTp[gpr] = g_pool.tile([P, 2, NQ], FP8, tag="g",
                                                   bufs=OT // 2,
                                                   name=f"g{gpr}")
                        g_t = gTp[gpr]
                        for qc in range(NQ // 512):
                            ps = epsum.tile([P, 512], F32, tag="mm1")
                            for pr in range(2):
                                nc.tensor.matmul(
                                    ps,
                                    lhsT=w1all[:, pr, :,
                                               ot * P:(ot + 1) * P],
                                    rhs=z2f8[:, pr, :,
                                             qc * 512:(qc + 1) * 512],
                                    start=(pr == 0), stop=(pr == 1),
                                    perf_mode=DR)
                            # fused rescale + bias + gelu out of PSUM on ACT
                            nc.scalar.activation(
                                out=g_t[:, gj, qc * 512:(qc + 1) * 512],
                                in_=ps, func=Act.Gelu,
                                bias=b1cols[ot], scale=1.0 / WS)
                    if gj == 1:
                        for w in range(WAVE):
                            tq = half * WAVE + w
                            nc.tensor.matmul(
                                f2ps[w],
                                lhsT=gTp[gpr][:, :, tq * P:(tq + 1) * P],
                                rhs=w2all[:, gpr, :, :],
                                start=(gpr == 0), stop=False, perf_mode=DR)
                for w in range(WAVE):
                    tq = half * WAVE + w
                    nc.tensor.matmul(f2ps[w], lhsT=ones2, rhs=b2row,
                                     start=False, stop=True)
                    o_t = out_pool.tile([P, C], F32, tag="out")
                    nc.vector.scalar_tensor_tensor(
                        out=o_t, in0=f2ps[w], scalar=1.0 / WS, in1=x2[tq],
                        op0=Alu.mult, op1=Alu.add)
                    nc.sync.dma_start(out=out[tq * P:(tq + 1) * P, :], in_=o_t)


# ---------------------------------------------------------------------------
# Host side
# ---------------------------------------------------------------------------

def _hi_lo(b):
    """Split fp32 row vector into bf16 hi + lo rows (hi + lo ~= b in fp32)."""
    import ml_dtypes
    b = np.asarray(b, np.float32)
    hi = b.astype(ml_dtypes.bfloat16)
    lo = (b - hi.astype(np.float32)).astype(ml_dtypes.bfloat16)
    return np.ascontiguousarray(np.stack([hi, lo], axis=0))


def _pack_dr(wT, np_fp8):
    """Pack a pre-transposed weight [Cin, Cout] (rows = contraction) into the
    fp8 DoubleRow layout [128, Cin//256, 2, Cout], pre-scaled by WS."""
    cin, cout = wT.shape
    pairs = cin // 256
    arr = (wT * WS).reshape(pairs, 2, P, cout).transpose(2, 0, 1, 3)
    return np.ascontiguousarray(arr.astype(np_fp8))


def prepare_inputs(x, qkv_w, proj_w, proj_b, rpb_table, n1_w, n1_b, n2_w, n2_b,
                   fc1_w, fc1_b, fc2_w, fc2_b):
    """Fold LN affines into weights, pre-transpose, build shifted bias blocks,
    and produce the 8 per-core input maps."""
    import ml_dtypes
    f = np.float32
    bf = ml_dtypes.bfloat16
    np_fp8 = mybir.dt.np(FP8)
    x = np.asarray(x, f)
    qkv_w = np.asarray(qkv_w, f)
    proj_w = np.asarray(proj_w, f)
    rpb = np.asarray(rpb_table, f)
    fc1_w = np.asarray(fc1_w, f)
    fc2_w = np.asarray(fc2_w, f)
    n1_w = np.asarray(n1_w, f); n1_b = np.asarray(n1_b, f)
    n2_w = np.asarray(n2_w, f); n2_b = np.asarray(n2_b, f)

    wqkvT = _pack_dr((qkv_w * n1_w[None, :]).T.astype(f), np_fp8)
    bqkv = (qkv_w @ n1_b).astype(f)
    wprojT = _pack_dr(proj_w.T.astype(f), np_fp8)
    wfc1T = _pack_dr((fc1_w * n2_w[None, :]).T.astype(f), np_fp8)
    bfc1x = (np.asarray(fc1_b, f) + fc1_w @ n2_b).astype(f)
    wfc2T = _pack_dr(fc2_w.T.astype(f), np_fp8)

    # bias blocks: value at (k-tile kt, partition p, own-query j) must be
    # rpb[k_glob - q_glob + N-1, h]; with own-first rolled rows and the view
    # i = j + (7 - kt%8)*128,
    #   half A (kt 0..7):  idx = 2943 + p - i
    #   half B (kt 8..15): idx = 3967 - 2048*parity + p - i
    # blocks hold 8*bias in bf16 (added to raw scores pre-softmax; the 1/8
    # scale is applied inside the exp activation)
    ii = np.arange(BLKW)[None, :]
    pp = np.arange(P)[:, None]
    idx_a = 2943 + pp - ii
    scale8 = float(DH) ** 0.5

    def _blk(idx):
        raw = rpb[idx, :].transpose(2, 0, 1)        # [H, P, BLKW]
        blk = raw * scale8
        for h in M_HEADS:
            blk[h] = np.exp(raw[h])
        return np.ascontiguousarray(blk.astype(bf))

    blka_np = _blk(idx_a)
    blkb_np = []
    for par in range(2):
        idx_b = 3967 - 2048 * par + pp - ii
        blkb_np.append(_blk(idx_b))

    shared = dict(
        wqkvT=wqkvT,
        bqk=np.ascontiguousarray(bqkv[:2 * C]),
        bv2=_hi_lo(WS * bqkv[2 * C:]),
        wprojT=wprojT,
        bproj2=_hi_lo(WS * np.asarray(proj_b, f)),
        wfc1T=wfc1T, bfc1=bfc1x, wfc2T=wfc2T,
        bfc22=_hi_lo(WS * np.asarray(fc2_b, f)),
        blka=blka_np,
    )
    in_maps = []
    for core in range(8):
        b, par = core // 2, core % 2
        xb_c = np.ascontiguousarray(np.roll(x[b], -par * NQ, axis=0))
        m = dict(shared)
        m["xb"] = xb_c
        m["blkb"] = blkb_np[par]
        in_maps.append(m)
    return in_maps


def assemble_output(results):
    out = np.empty((B, N, C), np.float32)
    for core in range(8):
        b, par = core // 2, core % 2
        out[b, par * NQ:(par + 1) * NQ, :] = results[core]["out"]
    return out


_cache = threading.local()


def _get_program():
    nc = getattr(_cache, "nc", None)
    if nc is None:
        nc = build_program(reps=1)
        _cache.nc = nc
    return nc


def kernel(**inputs) -> np.ndarray:
    in_maps = prepare_inputs(**inputs)
    nc = _get_program()
    res = run_bass_kernel_spmd(nc, in_maps, list(range(8)))
    return assemble_output(res.results)


if __name__ == "__main__":
    rng = np.random.default_rng(0)
    ins = {
        "x": rng.standard_normal((B, N, C)).astype(np.float32),
        "qkv_w": (rng.standard_normal((3 * C, C)) * 0.02).astype(np.float32),
        "proj_w": (rng.standard_normal((C, C)) * 0.02).astype(np.float32),
        "proj_b": np.zeros(C, np.float32),
        "rpb_table": (rng.standard_normal((2 * N - 1, H)) * 0.02).astype(np.float32),
        "n1_w": np.ones(C, np.float32), "n1_b": np.zeros(C, np.float32),
        "n2_w": np.ones(C, np.float32), "n2_b": np.zeros(C, np.float32),
        "fc1_w": (rng.standard_normal((HID, C)) * 0.02).astype(np.float32),
        "fc1_b": rng.standard_normal(HID).astype(np.float32),
        "fc2_w": (rng.standard_normal((C, HID)) * 0.02).astype(np.float32),
        "fc2_b": rng.standard_normal(C).astype(np.float32),
    }
    out = kernel(**ins)
    print("out", out.shape, out.dtype, float(np.abs(out).mean()))
